# revision 3
# baseline (speedup 1.0000x reference)
"""Sparse-attention ("Castle") Trainium2 kernel, 8-core SPMD.

Sharding: core c handles batch b = c // 4 and head pair p = c % 4
(heads 2p, 2p+1). Per core: project x[b] with this pair's W_qkv slice
(feature-major), build masked term1^T / lookahead^T (fp16), the N^3
Su contraction as block-sparse fp16 matmuls, softmax-free-of-max via
exp + column-sum, attention-weighted vc, and this head-pair's partial
of the output projection. Host sums the 4 partials per batch.

All matmuls run as float32r (TF32-like, ~1.5e-4 rel) except the big Su
contraction and attention@vc, which run fp16 from fp16-stored tiles.
"""
import sys
sys.path.insert(0, "/opt/trn_rl_repo")
import numpy as np

B, N, D = 2, 2048, 1024
HEADS, DH = 8, 64
NT = N // 128          # 16 row tiles
NC4 = NT // 4          # 4 chunks of 512
SCALE = DH ** -0.5

_STATE = {}


def _build_nc():
    import concourse.bacc as bacc
    import concourse.mybir as mybir
    from concourse import tile

    F32 = mybir.dt.float32
    F32R = mybir.dt.float32r
    F16 = mybir.dt.float16
    AF = mybir.ActivationFunctionType

    nc = bacc.Bacc("TRN2", target_bir_lowering=False, debug=False)

    xT = nc.dram_tensor("xT", [D, N], F32R, kind="ExternalInput")
    wqT = nc.dram_tensor("wqT", [D, 768], F32R, kind="ExternalInput")
    woT0 = nc.dram_tensor("woT0", [64, D], F32R, kind="ExternalInput")
    woT1 = nc.dram_tensor("woT1", [64, D], F32R, kind="ExternalInput")
    maskP = nc.dram_tensor("maskP", [128, 2048], F16, kind="ExternalInput")
    maskLT = nc.dram_tensor("maskLT", [128, 128], F16, kind="ExternalInput")
    ident = nc.dram_tensor("ident", [128, 128], F32R, kind="ExternalInput")
    onescol = nc.dram_tensor("onescol", [128, 8], F16, kind="ExternalInput")
    y = nc.dram_tensor("y", [N, D], F32, kind="ExternalOutput")

    VCW = 66  # stride of one vcA block (64 vc cols + 1 ones + pad)

    with tile.TileContext(nc) as tc:
        with tc.tile_pool(name="const", bufs=1) as cst, \
             tc.tile_pool(name="qkv", bufs=1) as qkvp, \
             tc.tile_pool(name="vca", bufs=1) as vcap, \
             tc.tile_pool(name="otn", bufs=1) as otnp:

            psAB = tc.alloc_tile_pool(name="psA", bufs=2, space="PSUM")
            psA = psAB
            mP = cst.tile([128, 2048], F16, tag="mP")
            mLT = cst.tile([128, 128], F16, tag="mLT")
            idn = cst.tile([128, 128], F32R, tag="idn")
            ones = cst.tile([128, 8], F16, tag="ones")
            wo0 = cst.tile([64, D], F32R, tag="wo0")
            wo1 = cst.tile([64, D], F32R, tag="wo1")
            nc.sync.dma_start(out=mP[:], in_=maskP.ap())
            nc.sync.dma_start(out=mLT[:], in_=maskLT.ap())
            nc.sync.dma_start(out=idn[:], in_=ident.ap())
            nc.sync.dma_start(out=ones[:], in_=onescol.ap())
            nc.sync.dma_start(out=wo0[:], in_=woT0.ap())
            nc.sync.dma_start(out=wo1[:], in_=woT1.ap())

            qk = [qkvp.tile([128, N], F32R, tag=f"qk{j}", name=f"qk{j}") for j in range(6)]

            # ---- Phase A: QKV projection (feature-major) ----
            with tc.tile_pool(name="xw", bufs=1) as xw:
                xt = []
                wt = []
                for dtile in range(8):
                    xti = xw.tile([128, N], F32R, tag=f"x{dtile}")
                    nc.sync.dma_start(
                        out=xti[:], in_=xT.ap()[dtile * 128:(dtile + 1) * 128, :]
                    )
                    xt.append(xti)
                    wti = xw.tile([128, 768], F32R, tag=f"w{dtile}")
                    nc.sync.dma_start(
                        out=wti[:], in_=wqT.ap()[dtile * 128:(dtile + 1) * 128, :]
                    )
                    wt.append(wti)
                for nch in range(4):
                    nsl = slice(nch * 512, nch * 512 + 512)
                    for j in range(6):
                        pp = psA.tile([128, 512], F32, tag="pr")
                        for dtile in range(8):
                            nc.tensor.matmul(
                                pp[:],
                                wt[dtile][:, j * 128:(j + 1) * 128],
                                xt[dtile][:, nsl],
                                start=(dtile == 0),
                                stop=(dtile == 7),
                            )
                        scl = SCALE if j in (0, 3) else 1.0
                        nc.vector.tensor_scalar_mul(qk[j][:, nsl], pp[:], scl)

            # ---- Phase B: vc -> n-major fp16 (+ ones col), both heads ----
            vca = [vcap.tile([128, VCW * NT], F16, tag=f"vca{h}", name=f"vca{h}") for h in range(2)]
            for kt in range(NT):
                pt = psA.tile([128, 128], F32R, tag="tr")
                nc.tensor.transpose(pt[:], qk[5][:, kt * 128:(kt + 1) * 128], idn[:])
                for h in range(2):
                    nc.vector.tensor_copy(
                        vca[h][:, kt * VCW:kt * VCW + 64],
                        pt[:, h * 64:(h + 1) * 64].bitcast(F32),
                    )
                    nc.vector.tensor_copy(
                        vca[h][:, kt * VCW + 64:kt * VCW + 65], ones[:, 0:1]
                    )

            psAB.release()

            otn = [otnp.tile([64, N], F32R, tag=f"otn{h}", name=f"otn{h}") for h in range(2)]

            # ---- Phase C: per-head attention ----
            for h in range(2):
                lo = h * 64
                quT = qk[0][lo:lo + 64, :]
                kuT = qk[1][lo:lo + 64, :]
                vuT = qk[2][lo:lo + 64, :]
                qcT = qk[3][lo:lo + 64, :]
                kcT = qk[4][lo:lo + 64, :]

                with tc.tile_pool(name=f"t1t{h}", bufs=1) as t1p, \
                     tc.tile_pool(name=f"lt{h}", bufs=1) as ltp, \
                     tc.tile_pool(name=f"wk{h}", bufs=3) as wk, \
                     tc.tile_pool(name=f"psC{h}", bufs=2, space="PSUM") as psC:

                    # T1T[jt] = masked(term1).T rows, fp16, spans i in [128*jt, N)
                    t1t = []
                    for jt in range(NT):
                        L = N - 128 * jt
                        t1 = t1p.tile([128, L], F16, tag=f"t1_{jt}", name=f"t1_{h}_{jt}")
                        t1t.append(t1)
                        i0 = 128 * jt
                        for ic in range(i0, N, 512):
                            w = min(512, N - ic)
                            ps = psC.tile([128, 512], F32, tag="mk")
                            nc.tensor.matmul(
                                ps[:, 0:w],
                                vuT[:, jt * 128:(jt + 1) * 128],
                                qcT[:, ic:ic + w],
                                start=True, stop=True,
                            )
                            if ic == i0:
                                nc.vector.tensor_mul(
                                    t1[:, 0:128], ps[:, 0:128], mP[:, 0:128]
                                )
                                if w > 128:
                                    nc.vector.tensor_copy(
                                        t1[:, 128:w], ps[:, 128:w]
                                    )
                            else:
                                nc.vector.tensor_copy(
                                    t1[:, ic - i0:ic - i0 + w], ps[:, 0:w]
                                )

                    # LT[jt] = masked sigmoid(lookahead).T rows, spans k in [0, 128*(jt+1))
                    ltt = []
                    for jt in range(NT):
                        Lk = 128 * (jt + 1)
                        lt = ltp.tile([128, Lk], F16, tag=f"lt_{jt}", name=f"lt_{h}_{jt}")
                        ltt.append(lt)
                        for k0 in range(0, Lk, 512):
                            w = min(512, Lk - k0)
                            ps = psC.tile([128, 512], F32, tag="mk")
                            nc.tensor.matmul(
                                ps[:, 0:w],
                                kuT[:, jt * 128:(jt + 1) * 128],
                                quT[:, k0:k0 + w],
                                start=True, stop=True,
                            )
                            nc.scalar.activation(
                                lt[:, k0:k0 + w], ps[:, 0:w], AF.Sigmoid
                            )
                        nc.vector.tensor_mul(
                            lt[:, Lk - 128:Lk], lt[:, Lk - 128:Lk], mLT[:]
                        )

                    # Su / scores / P / OT
                    for c in range(4):
                        csl = slice(512 * c, 512 * c + 512)
                        pot = psC.tile([65, 512], F32, tag="ot")
                        tmax = 4 * c + 3
                        for t in range(tmax + 1):
                            psu = psC.tile([128, 512], F32, tag="su")
                            for jt in range(t, tmax + 1):
                                s_loc = 512 * c - 128 * jt
                                if s_loc >= 0:
                                    nc.tensor.matmul(
                                        psu[:],
                                        ltt[jt][:, 128 * t:128 * t + 128],
                                        t1t[jt][:, s_loc:s_loc + 512],
                                        start=(jt == t), stop=(jt == tmax),
                                    )
                                else:
                                    nc.tensor.matmul(
                                        psu[:, -s_loc:512],
                                        ltt[jt][:, 128 * t:128 * t + 128],
                                        t1t[jt][:, 0:512 + s_loc],
                                        start=(jt == t), stop=(jt == tmax),
                                    )
                            if t > 4 * c:
                                nc.vector.memset(
                                    psu[:, 0:128 * t - 512 * c], 0.0
                                )
                            psc = psC.tile([128, 512], F32, tag="sc")
                            nc.tensor.matmul(
                                psc[:],
                                kcT[:, 128 * t:128 * t + 128],
                                qcT[:, csl],
                                start=True, stop=True,
                            )
                            sil = wk.tile([128, 512], F32, tag="sil")
                            nc.scalar.activation(sil[:], psu[:], AF.Silu)
                            stt = wk.tile([128, 512], F32, tag="stt")
                            nc.vector.tensor_sub(stt[:], psc[:], sil[:])
                            pexp = wk.tile([128, 512], F16, tag="pexp")
                            nc.scalar.activation(pexp[:], stt[:], AF.Exp)
                            if t >= 4 * c:
                                s = t - 4 * c
                                nc.vector.tensor_mul(
                                    pexp[:], pexp[:], mP[:, 512 * s:512 * s + 512]
                                )
                            nc.tensor.matmul(
                                pot[:],
                                vca[h][:, VCW * t:VCW * t + 65],
                                pexp[:],
                                start=(t == 0), stop=(t == tmax),
                            )
                        rec = wk.tile([1, 512], F32, tag="rec")
                        nc.vector.reciprocal(rec[:], pot[64:65, :])
                        recb = wk.tile([64, 512], F32, tag="recb")
                        nc.gpsimd.partition_broadcast(recb[:], rec[:], channels=64)
                        nc.vector.tensor_mul(otn[h][:, csl], pot[0:64, :], recb[:])

            # ---- Phase D: output projection (partial over this head pair) ----
            with tc.tile_pool(name="yp", bufs=2) as yp, \
                 tc.tile_pool(name="psD", bufs=2, space="PSUM") as psD:
                for it in range(NT):
                    ysb = yp.tile([128, D], F32, tag="ysb")
                    for dc in range(2):
                        py = psD.tile([128, 512], F32, tag="y")
                        nc.tensor.matmul(
                            py[:],
                            otn[0][:, it * 128:(it + 1) * 128],
                            wo0[:, dc * 512:(dc + 1) * 512],
                            start=True, stop=False,
                        )
                        nc.tensor.matmul(
                            py[:],
                            otn[1][:, it * 128:(it + 1) * 128],
                            wo1[:, dc * 512:(dc + 1) * 512],
                            start=False, stop=True,
                        )
                        nc.vector.tensor_copy(
                            ysb[:, dc * 512:(dc + 1) * 512], py[:]
                        )
                    nc.sync.dma_start(
                        out=y.ap()[it * 128:(it + 1) * 128, :], in_=ysb[:]
                    )

    nc.compile()
    return nc


class _SpmdRunner:
    def __init__(self, nc, n_cores=8):
        import jax
        from jax.sharding import Mesh, PartitionSpec
        from jax.experimental.shard_map import shard_map
        import concourse.mybir as mybir
        from concourse import bass2jax
        from concourse.bass2jax import _bass_exec_p, install_neuronx_cc_hook

        install_neuronx_cc_hook()
        self.jax = jax
        self.nc = nc
        self.n_cores = n_cores
        partition_name = (
            nc.partition_id_tensor.name if nc.partition_id_tensor else None
        )
        in_names, out_names, out_avals = [], [], []
        for alloc in nc.m.functions[0].allocations:
            if not isinstance(alloc, mybir.MemoryLocationSet):
                continue
            name = alloc.memorylocations[0].name
            if alloc.kind == "ExternalInput":
                if name != partition_name:
                    in_names.append(name)
            elif alloc.kind == "ExternalOutput":
                out_names.append(name)
                out_avals.append(
                    jax.core.ShapedArray(
                        tuple(alloc.tensor_shape), mybir.dt.np(alloc.dtype)
                    )
                )
        if nc.dbg_addr is not None:
            assert not nc.dbg_callbacks
            in_names.append(nc.dbg_addr.name)
            self.dbg_name = nc.dbg_addr.name
        else:
            self.dbg_name = None
        self.in_names = list(in_names)
        self.out_names = out_names
        self.out_avals = out_avals

        all_in_names = list(in_names)
        if partition_name is not None:
            all_in_names.append(partition_name)

        def _body(*args):
            operands = list(args)
            if partition_name is not None:
                operands.append(bass2jax.partition_id_tensor())
            outs = _bass_exec_p.bind(
                *operands,
                out_avals=tuple(out_avals),
                in_names=tuple(all_in_names),
                out_names=tuple(out_names),
                lowering_input_output_aliases=(),
                sim_require_finite=True,
                sim_require_nnan=True,
                nc=nc,
            )
            return tuple(outs)

        devices = jax.devices()[:n_cores]
        assert len(devices) == n_cores
        self.mesh = Mesh(np.asarray(devices), ("core",))
        in_specs = (PartitionSpec("core"),) * len(in_names)
        out_specs = (PartitionSpec("core"),) * len(out_names)
        self.fn = jax.jit(
            shard_map(
                _body,
                mesh=self.mesh,
                in_specs=in_specs,
                out_specs=out_specs,
                check_rep=False,
            ),
            keep_unused=True,
        )
        self.in_sharding = jax.sharding.NamedSharding(
            self.mesh, PartitionSpec("core")
        )

    def put_inputs(self, in_maps):
        assert len(in_maps) == self.n_cores
        if self.dbg_name is not None:
            in_maps = [
                {**m, self.dbg_name: np.zeros((1, 2), np.uint32)} for m in in_maps
            ]
        args = []
        for name in self.in_names:
            cat = np.concatenate(
                [np.asarray(in_maps[c][name]) for c in range(self.n_cores)],
                axis=0,
            )
            args.append(self.jax.device_put(cat, self.in_sharding))
        return args

    def run(self, dev_args):
        outs = self.fn(*dev_args)
        self.jax.block_until_ready(outs)
        return outs

    def outputs_to_host(self, outs):
        res = []
        for c in range(self.n_cores):
            d = {}
            for i, name in enumerate(self.out_names):
                d[name] = np.asarray(outs[i]).reshape(
                    self.n_cores, *self.out_avals[i].shape
                )[c]
            res.append(d)
        return res

    def __call__(self, in_maps):
        return self.outputs_to_host(self.run(self.put_inputs(in_maps)))


def _get_state():
    if "runner" not in _STATE:
        nc = _build_nc()
        _STATE["nc"] = nc
        _STATE["runner"] = _SpmdRunner(nc, 8)
    return _STATE


def make_in_maps(x, W_qkv, W_out):
    x = np.asarray(x, dtype=np.float32)
    W_qkv = np.asarray(W_qkv, dtype=np.float32)
    W_out = np.asarray(W_out, dtype=np.float32)

    ar128 = np.arange(128)
    maskP = np.zeros((128, 2048), np.float16)
    for s in range(4):
        maskP[:, 512 * s:512 * (s + 1)] = (
            np.arange(512)[None, :] >= (128 * s + ar128[:, None])
        )
    maskLT = (ar128[None, :] < ar128[:, None]).astype(np.float16)
    ident = np.eye(128, dtype=np.float32)
    onescol = np.ones((128, 8), np.float16)

    xT = [np.ascontiguousarray(x[b].T) for b in range(B)]
    in_maps = []
    for c in range(8):
        b, p = c // 4, c % 4
        rows = []
        for qkv in range(6):
            for hl in range(2):
                h = 2 * p + hl
                rows.append(W_qkv[qkv * 512 + h * 64:qkv * 512 + h * 64 + 64, :])
        wq = np.concatenate(rows, axis=0)          # [768, D]
        wqT = np.ascontiguousarray(wq.T)           # [D, 768]
        woT0 = np.ascontiguousarray(W_out[:, 128 * p:128 * p + 64].T)
        woT1 = np.ascontiguousarray(W_out[:, 128 * p + 64:128 * p + 128].T)
        in_maps.append({
            "xT": xT[b], "wqT": wqT, "woT0": woT0, "woT1": woT1,
            "maskP": maskP, "maskLT": maskLT, "ident": ident,
            "onescol": onescol,
        })
    return in_maps


def kernel(x, W_qkv, W_out):
    st = _get_state()
    in_maps = make_in_maps(x, W_qkv, W_out)
    res = st["runner"](in_maps)
    out = np.zeros((B, N, D), np.float32)
    for c in range(8):
        out[c // 4] += res[c]["y"]
    return out


if __name__ == "__main__":
    rng = np.random.default_rng(0)
    x = rng.standard_normal((B, N, D)).astype(np.float32)
    W_qkv = (rng.standard_normal((6 * 512, D)) * 0.02).astype(np.float32)
    W_out = (rng.standard_normal((D, 512)) * 0.02).astype(np.float32)
    y = kernel(x, W_qkv, W_out)
    print("kernel ran, out shape", y.shape, "finite:", np.isfinite(y).all())


# revision 6
# speedup vs baseline: 1.0689x; 1.0689x over previous
"""Sparse-attention ("Castle") Trainium2 kernel, 8-core SPMD.

Sharding: core c handles batch b = c // 4 and head pair p = c % 4
(heads 2p, 2p+1). Per core: project x[b] with this pair's W_qkv slice
(feature-major), build masked term1^T / lookahead^T (fp16), the N^3
Su contraction as block-sparse fp16 matmuls, softmax-free-of-max via
exp + column-sum, attention-weighted vc, and this head-pair's partial
of the output projection. Host sums the 4 partials per batch.

All matmuls run as float32r (TF32-like, ~1.5e-4 rel) except the big Su
contraction and attention@vc, which run fp16 from fp16-stored tiles.
"""
import sys
sys.path.insert(0, "/opt/trn_rl_repo")
import numpy as np

B, N, D = 2, 2048, 1024
HEADS, DH = 8, 64
NT = N // 128          # 16 row tiles
NC4 = NT // 4          # 4 chunks of 512
SCALE = DH ** -0.5

_STATE = {}


def _build_nc():
    import concourse.bacc as bacc
    import concourse.mybir as mybir
    from concourse import tile

    F32 = mybir.dt.float32
    F32R = mybir.dt.float32r
    F16 = mybir.dt.float16
    AF = mybir.ActivationFunctionType

    nc = bacc.Bacc("TRN2", target_bir_lowering=False, debug=False)

    xT = nc.dram_tensor("xT", [D, N], F32R, kind="ExternalInput")
    wqT = nc.dram_tensor("wqT", [D, 768], F32R, kind="ExternalInput")
    woT0 = nc.dram_tensor("woT0", [64, D], F16, kind="ExternalInput")
    woT1 = nc.dram_tensor("woT1", [64, D], F16, kind="ExternalInput")
    maskP = nc.dram_tensor("maskP", [128, 2048], F16, kind="ExternalInput")
    maskLT = nc.dram_tensor("maskLT", [128, 128], F16, kind="ExternalInput")
    ident = nc.dram_tensor("ident", [128, 128], F32R, kind="ExternalInput")
    onescol = nc.dram_tensor("onescol", [128, 8], F16, kind="ExternalInput")
    y = nc.dram_tensor("y", [N, D], F32, kind="ExternalOutput")

    VCW = 66  # stride of one vcA block (64 vc cols + 1 ones + pad)

    with tile.TileContext(nc) as tc:
        with tc.tile_pool(name="const", bufs=1) as cst, \
             tc.tile_pool(name="qkv", bufs=1) as qkvp, \
             tc.tile_pool(name="vca", bufs=1) as vcap, \
             tc.tile_pool(name="otn", bufs=1) as otnp:

            psAB = tc.alloc_tile_pool(name="psA", bufs=2, space="PSUM")
            psA = psAB
            mP = cst.tile([128, 2048], F16, tag="mP")
            mLT = cst.tile([128, 128], F16, tag="mLT")
            idn = cst.tile([128, 128], F32R, tag="idn")
            ones = cst.tile([128, 8], F16, tag="ones")
            wo0 = cst.tile([64, D], F16, tag="wo0")
            wo1 = cst.tile([64, D], F16, tag="wo1")
            nc.sync.dma_start(out=mP[:], in_=maskP.ap())
            nc.sync.dma_start(out=mLT[:], in_=maskLT.ap())
            nc.sync.dma_start(out=idn[:], in_=ident.ap())
            nc.sync.dma_start(out=ones[:], in_=onescol.ap())
            nc.sync.dma_start(out=wo0[:], in_=woT0.ap())
            nc.sync.dma_start(out=wo1[:], in_=woT1.ap())

            qk = [qkvp.tile([128, N], F16, tag=f"qk{j}", name=f"qk{j}") for j in range(5)]

            # ---- Phase A: QKV projection (feature-major) ----
            vca = [vcap.tile([128, VCW * NT], F16, tag=f"vca{h}", name=f"vca{h}") for h in range(2)]
            with tc.tile_pool(name="xw", bufs=1) as xw:
                vcT = xw.tile([128, N], F32R, tag="vcT", name="vcT")
                xt = []
                wt = []
                for dtile in range(8):
                    xti = xw.tile([128, N], F32R, tag=f"x{dtile}")
                    nc.sync.dma_start(
                        out=xti[:], in_=xT.ap()[dtile * 128:(dtile + 1) * 128, :]
                    )
                    xt.append(xti)
                    wti = xw.tile([128, 768], F32R, tag=f"w{dtile}")
                    nc.sync.dma_start(
                        out=wti[:], in_=wqT.ap()[dtile * 128:(dtile + 1) * 128, :]
                    )
                    wt.append(wti)
                for nch in range(4):
                    nsl = slice(nch * 512, nch * 512 + 512)
                    for j in range(6):
                        pp = psA.tile([128, 512], F32, tag="pr")
                        for dtile in range(8):
                            nc.tensor.matmul(
                                pp[:],
                                wt[dtile][:, j * 128:(j + 1) * 128],
                                xt[dtile][:, nsl],
                                start=(dtile == 0),
                                stop=(dtile == 7),
                            )
                        scl = SCALE if j in (0, 3) else 1.0
                        dst = vcT if j == 5 else qk[j]
                        nc.vector.tensor_scalar_mul(dst[:, nsl], pp[:], scl)

                # ---- Phase B: vc -> n-major fp16 (+ ones col), both heads ----
                for kt in range(NT):
                    pt = psA.tile([128, 128], F32R, tag="tr")
                    nc.tensor.transpose(pt[:], vcT[:, kt * 128:(kt + 1) * 128], idn[:])
                    for h in range(2):
                        nc.vector.tensor_copy(
                            vca[h][:, kt * VCW:kt * VCW + 64],
                            pt[:, h * 64:(h + 1) * 64].bitcast(F32),
                        )
                        nc.vector.tensor_copy(
                            vca[h][:, kt * VCW + 64:kt * VCW + 65], ones[:, 0:1]
                        )

            psAB.release()

            otn = [otnp.tile([64, N], F16, tag=f"otn{h}", name=f"otn{h}") for h in range(2)]

            # ---- Phase C: attention, both heads interleaved ----
            # One PSUM pool for everything (8 banks: mk2 su2 sc2 ot2) so the
            # scheduler can overlap head-1 builds with head-0 chunk loops.
            with tc.tile_pool(name="t1t", bufs=1) as t1p, \
                 tc.tile_pool(name="ltp", bufs=1) as ltp, \
                 tc.tile_pool(name="wk", bufs=3) as wk, \
                 tc.tile_pool(name="stp", bufs=1) as stp, \
                 tc.tile_pool(name="psC", bufs=2, space="PSUM") as psC:

                t1t = {}
                ltt = {}

                def build_head(h):
                    lo = h * 64
                    quT = qk[0][lo:lo + 64, :]
                    kuT = qk[1][lo:lo + 64, :]
                    vuT = qk[2][lo:lo + 64, :]
                    qcT = qk[3][lo:lo + 64, :]
                    # T1T rows
                    for jt in range(NT):
                        L = N - 128 * jt
                        t1 = t1p.tile([128, L], F16, tag=f"t1_{h}_{jt}",
                                      name=f"t1_{h}_{jt}")
                        t1t[(h, jt)] = t1
                        i0 = 128 * jt
                        for ic in range(i0, N, 512):
                            w = min(512, N - ic)
                            ps = psC.tile([128, 512], F32, tag="mk", name="mkp")
                            nc.tensor.matmul(
                                ps[:, 0:w],
                                vuT[:, jt * 128:(jt + 1) * 128],
                                qcT[:, ic:ic + w],
                                start=True, stop=True,
                            )
                            if ic == i0:
                                nc.vector.tensor_mul(
                                    t1[:, 0:128], ps[:, 0:128], mP[:, 0:128]
                                )
                                if w > 128:
                                    nc.vector.tensor_copy(t1[:, 128:w], ps[:, 128:w])
                            else:
                                nc.vector.tensor_copy(
                                    t1[:, ic - i0:ic - i0 + w], ps[:, 0:w]
                                )
                    # LT rows (sigmoid grouped on ACT)
                    for jt in range(NT):
                        Lk = 128 * (jt + 1)
                        lt = ltp.tile([128, Lk], F16, tag=f"lt_{h}_{jt}",
                                      name=f"lt_{h}_{jt}")
                        ltt[(h, jt)] = lt
                        for k0 in range(0, Lk, 512):
                            w = min(512, Lk - k0)
                            ps = psC.tile([128, 512], F32, tag="mk", name="mkp2")
                            nc.tensor.matmul(
                                ps[:, 0:w],
                                kuT[:, jt * 128:(jt + 1) * 128],
                                quT[:, k0:k0 + w],
                                start=True, stop=True,
                            )
                            nc.scalar.activation(lt[:, k0:k0 + w], ps[:, 0:w],
                                                 AF.Sigmoid)
                        nc.vector.tensor_mul(
                            lt[:, Lk - 128:Lk], lt[:, Lk - 128:Lk], mLT[:]
                        )

                def chunk_loop(h, c):
                    qcT = qk[3][h * 64:h * 64 + 64, :]
                    kcT = qk[4][h * 64:h * 64 + 64, :]
                    csl = slice(512 * c, 512 * c + 512)
                    tmax = 4 * c + 3
                    pot = psC.tile([65, 512], F32, tag="ot", name="pot")
                    for g0 in range(0, tmax + 1, 8):
                        g1 = min(g0 + 8, tmax + 1)
                        stts = {}
                        # pass 1: Su, Sc, silu (grouped), stt
                        for t in range(g0, g1):
                            psu = psC.tile([128, 512], F32, tag="su", name="psu")
                            for jt in range(t, tmax + 1):
                                s_loc = 512 * c - 128 * jt
                                if s_loc >= 0:
                                    nc.tensor.matmul(
                                        psu[:],
                                        ltt[(h, jt)][:, 128 * t:128 * t + 128],
                                        t1t[(h, jt)][:, s_loc:s_loc + 512],
                                        start=(jt == t), stop=(jt == tmax),
                                    )
                                else:
                                    nc.tensor.matmul(
                                        psu[:, -s_loc:512],
                                        ltt[(h, jt)][:, 128 * t:128 * t + 128],
                                        t1t[(h, jt)][:, 0:512 + s_loc],
                                        start=(jt == t), stop=(jt == tmax),
                                    )
                            if t > 4 * c:
                                nc.vector.memset(psu[:, 0:128 * t - 512 * c], 0.0)
                            psc = psC.tile([128, 512], F32, tag="sc", name="psc")
                            nc.tensor.matmul(
                                psc[:],
                                kcT[:, 128 * t:128 * t + 128],
                                qcT[:, csl],
                                start=True, stop=True,
                            )
                            sil = wk.tile([128, 512], F16, tag="sil", name="sil",
                                          bufs=2)
                            nc.scalar.activation(sil[:], psu[:], AF.Silu)
                            stt = stp.tile([128, 512], F16, tag=f"st{t - g0}",
                                           name="stt")
                            nc.vector.tensor_sub(stt[:], psc[:], sil[:])
                            stts[t] = stt
                        # pass 2: exp (grouped), mask, OT accumulation
                        for t in range(g0, g1):
                            pexp = wk.tile([128, 512], F16, tag="pexp",
                                           name="pexp", bufs=2)
                            nc.scalar.activation(pexp[:], stts[t][:], AF.Exp)
                            if t >= 4 * c:
                                s = t - 4 * c
                                nc.vector.tensor_mul(
                                    pexp[:], pexp[:], mP[:, 512 * s:512 * s + 512]
                                )
                            nc.tensor.matmul(
                                pot[:],
                                vca[h][:, VCW * t:VCW * t + 65],
                                pexp[:],
                                start=(t == 0), stop=(t == tmax),
                            )
                    rec = wk.tile([1, 512], F32, tag="rec", name="rec", bufs=1)
                    nc.vector.reciprocal(rec[:], pot[64:65, :])
                    recb = wk.tile([64, 512], F32, tag="recb", name="recb", bufs=1)
                    nc.gpsimd.partition_broadcast(recb[:], rec[:], channels=64)
                    nc.vector.tensor_mul(otn[h][:, csl], pot[0:64, :], recb[:])

                build_head(0)
                build_head(1)
                for c in range(4):
                    chunk_loop(0, c)
                for c in range(4):
                    chunk_loop(1, c)

            # ---- Phase D: output projection (partial over this head pair) ----
            with tc.tile_pool(name="yp", bufs=2) as yp, \
                 tc.tile_pool(name="psD", bufs=2, space="PSUM") as psD:
                for it in range(NT):
                    ysb = yp.tile([128, D], F32, tag="ysb")
                    for dc in range(2):
                        py = psD.tile([128, 512], F32, tag="y")
                        nc.tensor.matmul(
                            py[:],
                            otn[0][:, it * 128:(it + 1) * 128],
                            wo0[:, dc * 512:(dc + 1) * 512],
                            start=True, stop=False,
                        )
                        nc.tensor.matmul(
                            py[:],
                            otn[1][:, it * 128:(it + 1) * 128],
                            wo1[:, dc * 512:(dc + 1) * 512],
                            start=False, stop=True,
                        )
                        nc.vector.tensor_copy(
                            ysb[:, dc * 512:(dc + 1) * 512], py[:]
                        )
                    nc.sync.dma_start(
                        out=y.ap()[it * 128:(it + 1) * 128, :], in_=ysb[:]
                    )

    nc.compile()
    return nc


class _SpmdRunner:
    def __init__(self, nc, n_cores=8):
        import jax
        from jax.sharding import Mesh, PartitionSpec
        from jax.experimental.shard_map import shard_map
        import concourse.mybir as mybir
        from concourse import bass2jax
        from concourse.bass2jax import _bass_exec_p, install_neuronx_cc_hook

        install_neuronx_cc_hook()
        self.jax = jax
        self.nc = nc
        self.n_cores = n_cores
        partition_name = (
            nc.partition_id_tensor.name if nc.partition_id_tensor else None
        )
        in_names, out_names, out_avals = [], [], []
        for alloc in nc.m.functions[0].allocations:
            if not isinstance(alloc, mybir.MemoryLocationSet):
                continue
            name = alloc.memorylocations[0].name
            if alloc.kind == "ExternalInput":
                if name != partition_name:
                    in_names.append(name)
            elif alloc.kind == "ExternalOutput":
                out_names.append(name)
                out_avals.append(
                    jax.core.ShapedArray(
                        tuple(alloc.tensor_shape), mybir.dt.np(alloc.dtype)
                    )
                )
        if nc.dbg_addr is not None:
            assert not nc.dbg_callbacks
            in_names.append(nc.dbg_addr.name)
            self.dbg_name = nc.dbg_addr.name
        else:
            self.dbg_name = None
        self.in_names = list(in_names)
        self.out_names = out_names
        self.out_avals = out_avals

        all_in_names = list(in_names)
        if partition_name is not None:
            all_in_names.append(partition_name)

        def _body(*args):
            operands = list(args)
            if partition_name is not None:
                operands.append(bass2jax.partition_id_tensor())
            outs = _bass_exec_p.bind(
                *operands,
                out_avals=tuple(out_avals),
                in_names=tuple(all_in_names),
                out_names=tuple(out_names),
                lowering_input_output_aliases=(),
                sim_require_finite=True,
                sim_require_nnan=True,
                nc=nc,
            )
            return tuple(outs)

        devices = jax.devices()[:n_cores]
        assert len(devices) == n_cores
        self.mesh = Mesh(np.asarray(devices), ("core",))
        in_specs = (PartitionSpec("core"),) * len(in_names)
        out_specs = (PartitionSpec("core"),) * len(out_names)
        self.fn = jax.jit(
            shard_map(
                _body,
                mesh=self.mesh,
                in_specs=in_specs,
                out_specs=out_specs,
                check_rep=False,
            ),
            keep_unused=True,
        )
        self.in_sharding = jax.sharding.NamedSharding(
            self.mesh, PartitionSpec("core")
        )

    def put_inputs(self, in_maps):
        assert len(in_maps) == self.n_cores
        if self.dbg_name is not None:
            in_maps = [
                {**m, self.dbg_name: np.zeros((1, 2), np.uint32)} for m in in_maps
            ]
        args = []
        for name in self.in_names:
            cat = np.concatenate(
                [np.asarray(in_maps[c][name]) for c in range(self.n_cores)],
                axis=0,
            )
            args.append(self.jax.device_put(cat, self.in_sharding))
        return args

    def run(self, dev_args):
        outs = self.fn(*dev_args)
        self.jax.block_until_ready(outs)
        return outs

    def outputs_to_host(self, outs):
        res = []
        for c in range(self.n_cores):
            d = {}
            for i, name in enumerate(self.out_names):
                d[name] = np.asarray(outs[i]).reshape(
                    self.n_cores, *self.out_avals[i].shape
                )[c]
            res.append(d)
        return res

    def __call__(self, in_maps):
        return self.outputs_to_host(self.run(self.put_inputs(in_maps)))


def _get_state():
    if "runner" not in _STATE:
        nc = _build_nc()
        _STATE["nc"] = nc
        _STATE["runner"] = _SpmdRunner(nc, 8)
    return _STATE


def make_in_maps(x, W_qkv, W_out):
    x = np.asarray(x, dtype=np.float32)
    W_qkv = np.asarray(W_qkv, dtype=np.float32)
    W_out = np.asarray(W_out, dtype=np.float32)

    ar128 = np.arange(128)
    maskP = np.zeros((128, 2048), np.float16)
    for s in range(4):
        maskP[:, 512 * s:512 * (s + 1)] = (
            np.arange(512)[None, :] >= (128 * s + ar128[:, None])
        )
    maskLT = (ar128[None, :] < ar128[:, None]).astype(np.float16)
    ident = np.eye(128, dtype=np.float32)
    onescol = np.ones((128, 8), np.float16)

    xT = [np.ascontiguousarray(x[b].T) for b in range(B)]
    in_maps = []
    for c in range(8):
        b, p = c // 4, c % 4
        rows = []
        for qkv in range(6):
            for hl in range(2):
                h = 2 * p + hl
                rows.append(W_qkv[qkv * 512 + h * 64:qkv * 512 + h * 64 + 64, :])
        wq = np.concatenate(rows, axis=0)          # [768, D]
        wqT = np.ascontiguousarray(wq.T)           # [D, 768]
        woT0 = np.ascontiguousarray(W_out[:, 128 * p:128 * p + 64].T).astype(np.float16)
        woT1 = np.ascontiguousarray(W_out[:, 128 * p + 64:128 * p + 128].T).astype(np.float16)
        in_maps.append({
            "xT": xT[b], "wqT": wqT, "woT0": woT0, "woT1": woT1,
            "maskP": maskP, "maskLT": maskLT, "ident": ident,
            "onescol": onescol,
        })
    return in_maps


def kernel(x, W_qkv, W_out):
    st = _get_state()
    in_maps = make_in_maps(x, W_qkv, W_out)
    res = st["runner"](in_maps)
    out = np.zeros((B, N, D), np.float32)
    for c in range(8):
        out[c // 4] += res[c]["y"]
    return out


if __name__ == "__main__":
    rng = np.random.default_rng(0)
    x = rng.standard_normal((B, N, D)).astype(np.float32)
    W_qkv = (rng.standard_normal((6 * 512, D)) * 0.02).astype(np.float32)
    W_out = (rng.standard_normal((D, 512)) * 0.02).astype(np.float32)
    y = kernel(x, W_qkv, W_out)
    print("kernel ran, out shape", y.shape, "finite:", np.isfinite(y).all())
